# revision 30
# baseline (speedup 1.0000x reference)
"""MoE expert FFN (swiglu) kernel for 8 trn2 NeuronCores.

Expert parallelism: 8 experts, one per core. Each core computes, for its
expert e:
    h   = x_e @ w1_e            # [2048, 2048] @ [2048, 2816]
    act = silu(h[:, :1408]) * h[:, 1408:]
    out = act @ w2_e            # [2048, 1408] @ [1408, 2048]

Tokens arrive pre-sorted by expert with equal counts (2048/expert), so
sharding is a static slice and the gather is a concat. No collectives.

v6 = final (483.7us measured; v1 baseline 511us). History: v2 (591us)
and v7 (575us) tripped the sticky P0 power state (PE 2.4->2.0GHz) by
running the PE near-dense while the 13.5MB weight/activation preload
still streamed at full DMA rate - the chip's power governor punishes
dense-PE + dense-DMA startup, so the ~50%-busy PE ramp below is kept
deliberately. v3/v4 hung on HW (N=512 warmup matmuls / ACT-ring
stores; CoreSim-clean, hardware-level).
  - w1 host-packed into per-pair column slabs [j, p, (s, k, c)] loaded in
    consumption order: pair j waits only its own 1MB slab instead of the
    whole 11.5MB (v1 stalled the PE ~28us at chunk-0 start).
  - x host-packed per chunk [c, p, (k, t)]: one contiguous 2MB DMA per
    chunk; chunk 0 in 4 pieces so the first matmul starts sooner.
  - Ring plan: ACT ring carries ONLY x0 (the startup-critical load);
    SP ring carries w1 pairs then x1/w2 (needed from ~90us) then out
    stores; SWDGE carries x2/x3 whose buffer-free waits would block a
    HWDGE ring. v2 streamed x1+w2 concurrently with the w1 pairs and
    starved them (8.7us stall).
  - NO PE warmup burst and no chunk-0 PSUM-fanout: both raise early PE
    density into the P0 trigger zone (v2/v7 lost ~85us to the 2.0GHz
    downclock). Cold-start HAM cost (~1.7us) is cheap insurance.
  - out stored bf16 (cast during PSUM->SBUF drain), halving store bytes;
    w2 as one wide tile (single DMA/semaphore); last m-tile drains
    n-outer so the tail is one copy + one 128KB store.

Device layout (bf16 compute, fp32 PSUM accumulation):
  mm1: inter[f, t] psum tiles; lhsT = w1 slab slice (stationary),
       rhs = x[k, t] (moving, N=512) -> no on-device transpose anywhere.
  swiglu: act_j = silu(ps_a)*ps_b via ACT(Silu) + DVE mul -> bf16 SBUF.
  mm2: out[t, h]; lhsT = act 128-col slice, rhs = w2[f, h] (moving 512).

PE floor: 2112 matmuls x 215.8ns = 456us at 2.4GHz.
"""

import os
import sys

sys.path.insert(0, "/opt/trn_rl_repo")

import numpy as np
import ml_dtypes

E = 8             # experts == cores
T_TOTAL = 16384
H = 2048
F = 1408
F2 = 2 * F        # 2816
TPC = T_TOTAL // E  # 2048 tokens per core
CHUNK = 512
NCH = TPC // CHUNK          # 4 chunks
KH = H // 128               # 16 contraction tiles for mm1
NF = F // 128               # 11 f-blocks per half (a / b)
NT = CHUNK // 128           # 4 m-tiles per chunk in mm2
NHO = H // 512              # 4 output column blocks
XW = KH * CHUNK             # 8192 x-chunk tile width (k, t)
W1W = 2 * KH * 128          # 4096 w1 pair tile width (s, k, c)
ACT_FN = "Silu"             # swap to "Copy" for CoreSim (no Silu there)

_CACHE = {}

# Optional knobs read by test.py (not used by the grading harness).
TRACE = os.environ.get("BASS_TRACE_KERNEL", "0") == "1"
LAST = {}


def _build():
    from concourse import bacc, tile, mybir

    bf16 = mybir.dt.bfloat16
    f32 = mybir.dt.float32
    SILU = getattr(mybir.ActivationFunctionType, ACT_FN)

    # Bacc (not plain Bass): its lowering pipeline splits multi-sem waits
    # into EventSemaphore pairs — TRN2 allows at most 1 wait per instruction.
    nc = bacc.Bacc()
    x_d = nc.declare_dram_parameter("x", [NCH * 128, XW], bf16, isOutput=False)
    w1_d = nc.declare_dram_parameter("w1", [NF * 128, W1W], bf16, isOutput=False)
    w2_d = nc.declare_dram_parameter("w2", [128, NF * H], bf16, isOutput=False)
    out_d = nc.declare_dram_parameter("out", [TPC, H], bf16, isOutput=True)

    with tile.TileContext(nc) as tc:
        with (
            tc.tile_pool(name="w1p", bufs=1) as w1p,
            tc.tile_pool(name="w2p", bufs=1) as w2p,
            tc.tile_pool(name="xp", bufs=2) as xp,
            tc.tile_pool(name="actp", bufs=1) as actp,
            tc.tile_pool(name="tmpp", bufs=2) as tmpp,
            tc.tile_pool(name="outp", bufs=2) as outp,
            tc.tile_pool(name="psp", bufs=8, space="PSUM") as psp,
        ):
            # --- x chunk tiles; chunk 0 in 4 ascending pieces on the
            # otherwise-empty ACT ring (startup-critical: pair-0's k-sweep
            # consumes k-ascending; a small first piece starts the PE ~2us
            # sooner, receipts of later bigger pieces pipeline).
            x_t = []
            for c in range(NCH):
                x_t.append(xp.tile([128, XW], bf16, tag="xc", name=f"x_{c}"))
            for lo, hi in ((0, 1024), (1024, 2048), (2048, 5120), (5120, 8192)):
                nc.scalar.dma_start(
                    out=x_t[0][:, lo:hi],
                    in_=x_d[0:128, lo:hi],
                )

            # --- w1 pair slabs on the SP ring, in consumption order.
            # Pair 0 split k-ascending so its first matmuls arrive sooner.
            w1_t = []
            for j in range(NF):
                w1_t.append(w1p.tile([128, W1W], bf16, tag=f"w1_{j}", name=f"w1_{j}"))
            for lo, hi in ((0, 512), (512, 1024), (1024, 2048), (2048, 4096)):
                nc.sync.dma_start(out=w1_t[0][:, lo:hi], in_=w1_d[0:128, lo:hi])
            for j in range(1, NF):
                nc.sync.dma_start(out=w1_t[j][:], in_=w1_d[j * 128 : (j + 1) * 128, :])

            # --- x1 and w2 also on the SP ring AFTER all w1 pairs: they are
            # not needed until ~90us, and streaming them early (v2) starved
            # the w1 pair stream the PE was waiting on. w2 is one wide tile
            # loaded by a single 5.8MB DMA: one completion semaphore instead
            # of 11, so mm2's first chunk pays one PE wait instead of 11.
            nc.sync.dma_start(out=x_t[1][:], in_=x_d[128:256, :])
            w2t = w2p.tile([128, NF * H], bf16, tag="w2", name="w2t")
            nc.sync.dma_start(out=w2t[:], in_=w2_d[:, :])

            # --- x2/x3 on SWDGE: their buffer-free waits (x0/x1 reuse)
            # would block a HWDGE ring; gpsimd has nothing else queued.
            nc.gpsimd.dma_start(out=x_t[2][:], in_=x_d[256:384, :])
            nc.gpsimd.dma_start(out=x_t[3][:], in_=x_d[384:512, :])

            for c in range(NCH):
                xc = x_t[c]

                # mm1 + swiglu, one (a, b) f-block pair at a time.
                act_t = []
                for j in range(NF):
                    ps_a = psp.tile([128, CHUNK], f32, tag="ps")
                    ps_b = psp.tile([128, CHUNK], f32, tag="ps")
                    for k in range(KH):
                        nc.tensor.matmul(
                            ps_a[:],
                            w1_t[j][:, k * 128 : (k + 1) * 128],
                            xc[:, k * CHUNK : (k + 1) * CHUNK],
                            start=(k == 0),
                            stop=(k == KH - 1),
                        )
                    for k in range(KH):
                        nc.tensor.matmul(
                            ps_b[:],
                            w1_t[j][:, 2048 + k * 128 : 2048 + (k + 1) * 128],
                            xc[:, k * CHUNK : (k + 1) * CHUNK],
                            start=(k == 0),
                            stop=(k == KH - 1),
                        )
                    tmp = tmpp.tile([128, CHUNK], f32, tag="tmp")
                    nc.scalar.activation(tmp[:], ps_a[:], SILU)
                    a = actp.tile([128, CHUNK], bf16, tag=f"act_{j}")
                    act_t.append(a)
                    nc.vector.tensor_mul(a[:], tmp[:], ps_b[:])

                # mm2: out[t, h] for this chunk; bf16 staging.
                for m in range(NT):
                    last = (c == NCH - 1 and m == NT - 1)
                    po = [
                        psp.tile([128, 512], f32, tag="ps", name=f"po_{c}_{m}_{n}")
                        for n in range(NHO)
                    ]
                    osb = outp.tile([128, H], bf16, tag="osb")
                    r0 = c * CHUNK + m * 128
                    if not last:
                        # n-inner: the 4 psum tiles retire within ~850ns of
                        # each other; one copy batch + one 512KB store.
                        for k in range(NF):
                            lhsT = act_t[k][:, m * 128 : (m + 1) * 128]
                            for n in range(NHO):
                                nc.tensor.matmul(
                                    po[n][:],
                                    lhsT,
                                    w2t[:, k * H + n * 512 : k * H + (n + 1) * 512],
                                    start=(k == 0),
                                    stop=(k == NF - 1),
                                )
                        for n in range(NHO):
                            nc.scalar.copy(osb[:, n * 512 : (n + 1) * 512], po[n][:])
                        nc.sync.dma_start(out=out_d[r0 : r0 + 128, :], in_=osb[:])
                    else:
                        # last m-tile n-outer: po[n] retires after 11 matmuls,
                        # so its copy+128KB store overlap po[n+1]'s matmuls —
                        # the kernel tail is one copy + one small store
                        # instead of 4 copies + a 512KB store.
                        for n in range(NHO):
                            for k in range(NF):
                                nc.tensor.matmul(
                                    po[n][:],
                                    act_t[k][:, m * 128 : (m + 1) * 128],
                                    w2t[:, k * H + n * 512 : k * H + (n + 1) * 512],
                                    start=(k == 0),
                                    stop=(k == NF - 1),
                                )
                            nc.scalar.copy(osb[:, n * 512 : (n + 1) * 512], po[n][:])
                            nc.sync.dma_start(
                                out=out_d[r0 : r0 + 128, n * 512 : (n + 1) * 512],
                                in_=osb[:, n * 512 : (n + 1) * 512],
                            )
    if not nc.is_finalized():
        nc.finalize()  # Bacc.finalize runs the lowering pipeline (sem split, alloc_regs)
    return nc


def _get_nc():
    if "nc" not in _CACHE:
        _CACHE["nc"] = _build()
    return _CACHE["nc"]


def kernel(permuted_hidden_states, num_tokens_per_expert, w1, w2):
    from concourse.bass_utils import run_bass_kernel_spmd

    x = np.asarray(permuted_hidden_states, dtype=np.float32)
    w1 = np.asarray(w1, dtype=np.float32)
    w2 = np.asarray(w2, dtype=np.float32)
    ntpe = np.asarray(num_tokens_per_expert)
    assert x.shape == (T_TOTAL, H) and w1.shape == (E, H, F2) and w2.shape == (E, F, H)
    # Reference semantics rely on the static equal split.
    assert np.all(ntpe == TPC), f"expected equal {TPC}-token splits, got {ntpe}"

    bf = ml_dtypes.bfloat16
    in_maps = []
    for e in range(E):
        xe = x[e * TPC : (e + 1) * TPC].astype(bf)
        # x: [c, p, (k, t)] — chunk-major, partition = hidden row within k-tile
        xr = np.ascontiguousarray(
            xe.reshape(NCH, CHUNK, KH, 128).transpose(0, 3, 2, 1)
        ).reshape(NCH * 128, XW)
        w1e = w1[e].astype(bf)
        # w1: [j, p, (s, k, c)] — pair-major slabs; a-half cols 0:2048,
        # b-half cols 2048:4096, each (k, c) ordered.
        A = w1e[:, :F].reshape(KH, 128, NF, 128).transpose(2, 1, 0, 3)
        B = w1e[:, F:].reshape(KH, 128, NF, 128).transpose(2, 1, 0, 3)
        w1r = np.ascontiguousarray(
            np.stack([A, B], axis=2)
        ).reshape(NF * 128, W1W)
        # w2: [p, (k, c)] — one wide tile, partition = row within k-tile
        w2r = np.ascontiguousarray(
            w2[e].astype(bf).reshape(NF, 128, H).transpose(1, 0, 2)
        ).reshape(128, NF * H)
        in_maps.append(
            {
                "x": xr,
                "w1": w1r,
                "w2": w2r,
            }
        )

    nc = _get_nc()
    res = run_bass_kernel_spmd(nc, in_maps, list(range(E)), trace=TRACE)
    LAST["exec_time_ns"] = res.exec_time_ns
    LAST["mean_exec_time_ns"] = res.mean_exec_time_ns
    LAST["profile_json"] = res.profile_json
    out = np.concatenate([res.results[i]["out"] for i in range(E)], axis=0)
    return np.ascontiguousarray(out.astype(np.float32))
